# revision 25
# baseline (speedup 1.0000x reference)
"""Trainium2 Bass kernel for nn_Block_39513699123558 (gnn_message_passing).

Two layers of (Chebyshev graph conv K=5 -> BatchNorm -> ReLU) on
x[B=2, F0=16, V=162, X=Y=Z=16].

Strategy (8 NeuronCores, data-parallel over B x S-quarters):
  - each core owns shard [Fin, V, S=1024] (b = core//4, s-quarter = core%4)
  - s is processed in NCH=8 chunks of SC=128 columns
  - Chebyshev k=1..4 via one host-precomputed T-stack (shared by both
    layers, T_k symmetric); T_0 = identity handled without matmuls
  - conv: matmul contracting V (lhsT = T-stack tiles, rhs = activations
    [v, (f s)]) -> xs[(k u), (f s)]
  - layout bridge xs -> xsT[(k f), (u s)] goes through small DRAM staging
    buffers: per-tile span writes (gpsimd software DGE, cheap descriptors)
    + one contiguous read back per chunk.  This keeps the dma_start count
    ~30x below the per-row SBUF bridge (each dma_start costs ~0.6us on
    the issuing sequencer + HWDGE).
  - L1 projection: single matmul contracting (k f)=80 (identity rows of
    xsT filled from a host-transposed copy of x read straight from HBM).
    Vertex axis split in 4 bands of 42/42/42/36 on PE column strips.
  - L2 projection: 2 accumulated matmuls: k=1..4 from xsT2 (128 rows)
    + k=0 read directly from the normalized y1 slab (rhs partitions
    32j..32j+32, replicated w2[0] weights, tile_position (32j, 32j)).
  - projection psum tiles cover r=3 vertex rows x SC per band strip;
    bn_stats off each psum tile, count-weighted AllReduce at layer end,
    per-partition scale/shift + ReLU applied lazily (JIT) next pass.
  - proj of chunk c is emitted after conv of chunk c+1 so the DRAM
    read-back latency hides under conv; xsT pools stay single/double
    buffered within SBUF limits.
All matmul data bf16; PSUM/stats/normalization math f32; output f32.
"""

import os
import sys

sys.path.insert(0, "/opt/trn_rl_repo")

SKIP_CC = os.environ.get("K_SKIP_CC", "0") == "1"
CC_MODE = os.environ.get("K_CC", "both")  # none|l1|l2|both


import numpy as np
import ml_dtypes

from concourse import bass, bacc, mybir
from concourse import tile
from concourse.bass_utils import run_bass_kernel_spmd

BF16 = ml_dtypes.bfloat16
BF = mybir.dt.bfloat16
F32 = mybir.dt.float32

V = 162
VA = 128
VB = V - VA  # 34
F1, F2 = 16, 32
K = 5
S = 1024          # s-columns per core
SC = 128          # s-chunk
NCH = S // SC     # 8
EPS = 1e-5
N_CORES = 8

# vertex bands per PE column strip: u in [UB[j], UB[j+1])
UB = [0, 42, 84, 126, 162]
BW = [42, 42, 42, 36]
R = 3             # u-rows per proj psum slot
NG = 14           # slots per band (band 3 uses 12 of 14)
NGJ = [14, 14, 14, 12]
CW = 42 * SC      # yslab cols per chunk

# T-stack k=1..4 (T_0 = I handled separately), 648 rows, 6 tiles of 108
ST = [108] * 6
SP = [(1, 0, 0, 0, 108), (1, 1, 0, 108, 54),
      (2, 1, 54, 0, 54), (2, 2, 0, 54, 108),
      (3, 3, 0, 0, 108), (3, 4, 0, 108, 54),
      (4, 4, 54, 0, 54), (4, 5, 0, 54, 108)]
SP_BY_T = [[sp for sp in SP if sp[1] == t] for t in range(6)]

NSUB = R * SC                 # proj psum cols (f32)
NSLOT = NCH * NG              # 112 bn-stats slots (chunk, g)
STSCR_W = NSLOT * 8


def build_program():
    nc = bacc.Bacc("TRN2", target_bir_lowering=False)
    xk = nc.declare_dram_parameter("xk", [V, NCH, F1, SC], BF, False)
    xkt = nc.declare_dram_parameter("xkt", [NCH, F1, V * SC], BF, False)
    tsk = nc.declare_dram_parameter("tsk", [V, 648], BF, False)
    w1r = nc.declare_dram_parameter("w1r", [K * F1, F2], BF, False)
    w2k14 = nc.declare_dram_parameter("w2k14", [128, F2], BF, False)
    w2k0q = nc.declare_dram_parameter("w2k0q", [128, F2], BF, False)
    gb1 = nc.declare_dram_parameter("gb1", [128, 2], F32, False)
    gb2 = nc.declare_dram_parameter("gb2", [128, 2], F32, False)
    wrow = nc.declare_dram_parameter("wrow", [128, 1], F32, False)
    out = nc.declare_dram_parameter("out", [F2, V, S], F32, isOutput=True)

    with tile.TileContext(nc) as tc:
        with (
            tc.tile_pool(name="consts", bufs=1) as cpool,
            tc.tile_pool(name="slab", bufs=1) as slab,
            tc.tile_pool(name="stats", bufs=1) as spool,
            tc.tile_pool(name="dram", bufs=1, space="DRAM") as dram,
        ):
            tA = cpool.tile([VA, 648], BF)
            tB = cpool.tile([VB, 648], BF)
            w1t = cpool.tile([K * F1, F2], BF)
            w2at = cpool.tile([128, F2], BF)
            w2bt = cpool.tile([128, F2], BF)
            gb1t = cpool.tile([128, 2], F32)
            gb2t = cpool.tile([128, 2], F32)
            wrt = cpool.tile([128, 1], F32)
            nc.sync.dma_start(tA[:], tsk[0:VA, :])
            nc.sync.dma_start(tB[:], tsk[VA:V, :])
            nc.sync.dma_start(w1t[:], w1r[:])
            nc.sync.dma_start(w2at[:], w2k14[:])
            nc.sync.dma_start(w2bt[:], w2k0q[:])
            nc.sync.dma_start(gb1t[:], gb1[:])
            nc.sync.dma_start(gb2t[:], gb2[:])
            nc.sync.dma_start(wrt[:], wrow[:])

            # y-slab: rows 32j+o; cols per chunk c: (u-local 42, s SC)
            yslab = slab.tile([128, NCH * CW], BF)

            def ysl_c(c):
                return yslab[:, c * CW:(c + 1) * CW].rearrange(
                    "p (u s) -> p u s", u=42, s=SC)

            stscr1 = spool.tile([128, STSCR_W], F32)
            stscr2 = spool.tile([128, STSCR_W], F32)
            par1 = spool.tile([128, 2], F32)
            par2 = spool.tile([128, 2], F32)
            nc.gpsimd.memset(stscr1[:], 0.0)
            nc.gpsimd.memset(stscr2[:], 0.0)

            def proj_evac(c, stscr, mxps, mm_fn):
                # projection for chunk c: band j on column strip j; the
                # bands share one [128, NSUB] psum tile per u-slot g.
                # mm_fn(ps2, j, col0, c, wave) emits one matmul wave for a
                # strip; waves are emitted band-major so the 4 strips run
                # concurrently on the PE array.
                ysl = ysl_c(c)
                for g in range(NG):
                    rows = 128 if g < 12 else 96
                    nj = 4 if g < 12 else 3
                    ps2 = mxps.tile([128, NSUB], F32, tag="m2ps")
                    for wave in range(mm_fn.waves):
                        for j in range(nj):
                            col0 = (UB[j] + R * g) * SC
                            mm_fn(ps2, j, col0, c, wave)
                    dst = ysl[0:rows, R * g:R * g + R, :]
                    src = ps2[0:rows, :].rearrange("p (r s) -> p r s",
                                                   r=R, s=SC)
                    if g % 2 == 0:
                        nc.vector.tensor_copy(dst, src)
                    else:
                        nc.scalar.copy(dst, src)
                    # stats off the evacuated bf16 copy: frees the psum
                    # tile as soon as the copy retires (bn_stats free-dim
                    # limit is 512, so one call per g)
                    slot = c * NG + g
                    st = stscr[0:rows, slot * 8:slot * 8 + 6]
                    flat = yslab[0:rows,
                                 c * CW + R * g * SC:c * CW + (R * g + R) * SC]
                    nc.vector.bn_stats(st, flat)

            def conv_spans(c, rhs_pair, nf, xsp, xsd, m1ps, eng_flip):
                # graph-conv matmuls (k=1..4) + psum evac + DRAM span
                # writes for chunk c.  rhs_pair = (xa, xb) tiles
                # [V-part, nf*SC]; writes xsd rows (k-1)*nf+f.
                nspl = nf * SC // 512
                for m in range(6):
                    xst = xsp.tile([108, nf * SC], BF, tag="xst")
                    for spl in range(nspl):
                        ps = m1ps.tile([108, 512], F32, tag="m1ps")
                        for kc, (tt, xx) in enumerate(
                                ((tA, rhs_pair[0]), (tB, rhs_pair[1]))):
                            nc.tensor.matmul(
                                ps[:], tt[:, m * 108:(m + 1) * 108],
                                xx[:, spl * 512:(spl + 1) * 512],
                                start=(kc == 0), stop=(kc == 1))
                        dst = xst[:, spl * 512:(spl + 1) * 512]
                        if (m + spl + eng_flip) % 2 == 0:
                            nc.vector.tensor_copy(dst, ps[:])
                        else:
                            nc.scalar.copy(dst, ps[:])
                    for (k, t, r0, u0, span) in SP_BY_T[m]:
                        nc.gpsimd.dma_start(
                            xsd[(k - 1) * nf:k * nf,
                                u0 * SC:(u0 + span) * SC].rearrange(
                                "f (u s) -> u f s", u=span, s=SC),
                            xst[r0:r0 + span, :].rearrange(
                                "u (f s) -> u f s", f=nf, s=SC))

            def mm_l1(xsT):
                def mm(ps2, j, col0, c, wave):
                    nc.tensor.matmul(
                        ps2[32 * j:32 * j + 32, :], w1t[:],
                        xsT[:, col0:col0 + NSUB],
                        start=True, stop=True, tile_position=(0, 32 * j))
                mm.waves = 1
                return mm

            def mm_l2(xsT2):
                # wave 0: k=1..4 on the 4 column strips (concurrent);
                # wave 1: k=0 on 4 disjoint diagonal 32x32 tiles
                def mm(ps2, j, col0, c, wave):
                    if wave == 0:
                        nc.tensor.matmul(
                            ps2[32 * j:32 * j + 32, :], w2at[:],
                            xsT2[:, col0:col0 + NSUB],
                            start=True, stop=False,
                            tile_position=(0, 32 * j))
                    else:
                        g = (col0 // SC - UB[j]) // R
                        nc.tensor.matmul(
                            ps2[32 * j:32 * j + 32, :],
                            w2bt[32 * j:32 * j + 32, :],
                            ysl_c(c)[32 * j:32 * j + 32,
                                     R * g:R * g + R, :],
                            start=False, stop=True,
                            tile_position=(32 * j, 32 * j))
                mm.waves = 2
                return mm

            def bn_finalize(stscr, gbt, par, tag):
                # per-row (mean, var) -> count-weighted (E, S) -> AllReduce
                # -> band-fold -> scale/shift
                sv = stscr[:, :].rearrange("p (n e) -> p n e", n=NSLOT, e=8)
                mv = spool.tile([128, 2], F32, tag=f"mv{tag}")
                nc.vector.bn_aggr(mv[:], sv[:, :, 0:6])
                es = spool.tile([128, 2], F32, tag=f"es{tag}")
                nc.vector.tensor_mul(es[:, 1:2], mv[:, 0:1], mv[:, 0:1])
                nc.vector.tensor_add(es[:, 1:2], es[:, 1:2], mv[:, 1:2])
                nc.vector.tensor_copy(es[:, 0:1], mv[:, 0:1])
                nc.vector.tensor_mul(es[:, 0:1], es[:, 0:1], wrt[:, 0:1])
                nc.vector.tensor_mul(es[:, 1:2], es[:, 1:2], wrt[:, 0:1])
                cin = dram.tile([128, 2], F32, tag=f"cin{tag}")
                cout = dram.tile([128, 2], F32, tag=f"cout{tag}")
                nc.gpsimd.dma_start(cin[:], es[:])
                use_cc = (CC_MODE == "both" or CC_MODE == ("l" + tag)) \
                    and not SKIP_CC
                if use_cc:
                    nc.gpsimd.collective_compute(
                        "AllReduce", mybir.AluOpType.add,
                        replica_groups=[list(range(N_CORES))],
                        ins=[cin[:].opt()], outs=[cout[:].opt()])
                else:
                    nc.gpsimd.dma_start(cout[:], cin[:])
                qs = spool.tile([32, 8], F32, tag=f"qs{tag}")
                nc.sync.dma_start(
                    qs[:].rearrange("o (j e) -> o j e", j=4, e=2),
                    cout[:].rearrange("(j o) e -> o j e", j=4, o=32))
                acc = spool.tile([32, 6], F32, tag=f"acc{tag}")
                nc.vector.tensor_add(acc[:, 0:2], qs[:, 0:2], qs[:, 2:4])
                nc.vector.tensor_add(acc[:, 2:4], qs[:, 4:6], qs[:, 6:8])
                nc.vector.tensor_add(acc[:, 0:2], acc[:, 0:2], acc[:, 2:4])
                # acc[:,0]=global mean, acc[:,1]=global E[y^2]
                nc.vector.tensor_mul(acc[:, 2:3], acc[:, 0:1], acc[:, 0:1])
                nc.vector.tensor_sub(acc[:, 1:2], acc[:, 1:2], acc[:, 2:3])
                nc.vector.tensor_scalar_add(acc[:, 1:2], acc[:, 1:2], EPS)
                nc.scalar.sqrt(acc[:, 2:3], acc[:, 1:2])
                nc.vector.reciprocal(acc[:, 3:4], acc[:, 2:3])
                nc.vector.tensor_mul(acc[:, 4:5], gbt[0:32, 0:1], acc[:, 3:4])
                nc.vector.tensor_mul(acc[:, 5:6], acc[:, 0:1], acc[:, 4:5])
                nc.vector.tensor_sub(acc[:, 5:6], gbt[0:32, 1:2], acc[:, 5:6])
                for j in range(4):
                    nc.sync.dma_start(par[32 * j:32 * j + 32, 0:2],
                                      acc[:, 4:6])

            # ---- layer 1 ----
            with (
                tc.tile_pool(name="x", bufs=2) as xpool,
                tc.tile_pool(name="m1ps", bufs=3, space="PSUM") as m1ps,
                tc.tile_pool(name="m2ps", bufs=5, space="PSUM") as m2ps,
                tc.tile_pool(name="xs", bufs=3) as xsp,
                tc.tile_pool(name="xsT", bufs=2) as xtp,
                tc.tile_pool(name="xsd", bufs=2, space="DRAM") as xdp,
            ):
                def l1_head(c):
                    xa = xpool.tile([VA, F1 * SC], BF, tag="xa")
                    xb = xpool.tile([VB, F1 * SC], BF, tag="xb")
                    nc.sync.dma_start(xa[:], xk[0:VA, c, :, :])
                    nc.sync.dma_start(xb[:], xk[VA:V, c, :, :])
                    return xa, xb

                pend = []
                xcur = l1_head(0)
                for c in range(NCH):
                    xsd = xdp.tile([(K - 1) * F1, V * SC], BF, tag="xsd1")
                    conv_spans(c, xcur, F1, xsp, xsd, m1ps, 0)
                    if c + 1 < NCH:
                        xcur = l1_head(c + 1)
                    xsT = xtp.tile([K * F1, V * SC], BF, tag="xsT")
                    nc.sync.dma_start(xsT[0:F1, :], xkt[c, :, :])
                    nc.sync.dma_start(xsT[F1:K * F1, :], xsd[:, :])
                    # lag the projection 2 chunks so the DRAM read-back
                    # has a full conv window of slack
                    pend.append((c, xsT))
                    if len(pend) > 2:
                        pc, pxsT = pend.pop(0)
                        proj_evac(pc, stscr1, m2ps, mm_l1(pxsT))
                for pc, pxsT in pend:
                    proj_evac(pc, stscr1, m2ps, mm_l1(pxsT))
            bn_finalize(stscr1, gb1t, par1, "1")

            # ---- layer 2 ----
            with (
                tc.tile_pool(name="h1", bufs=2) as h1p,
                tc.tile_pool(name="m1ps2", bufs=3, space="PSUM") as m1ps2,
                tc.tile_pool(name="m2ps2", bufs=5, space="PSUM") as m2ps2,
                tc.tile_pool(name="xs2", bufs=3) as xsp2,
                tc.tile_pool(name="xsT2", bufs=1) as xtp2,
                tc.tile_pool(name="xsd2", bufs=2, space="DRAM") as xdp2,
                tc.tile_pool(name="h1d", bufs=2, space="DRAM") as hdp,
            ):
                def l2_head(c):
                    # JIT normalize+relu of chunk c (all bands), in place,
                    # then bridge to DRAM [u, (o s)] and load conv rhs
                    ysl = ysl_c(c)
                    nc.scalar.activation(
                        yslab[:, c * CW:(c + 1) * CW],
                        yslab[:, c * CW:(c + 1) * CW],
                        mybir.ActivationFunctionType.Relu,
                        bias=par1[:, 1:2], scale=par1[:, 0:1])
                    h1d = hdp.tile([V, F2 * SC], BF, tag="h1d")
                    for j in range(4):
                        bw = UB[j + 1] - UB[j]
                        nc.gpsimd.dma_start(
                            h1d[UB[j]:UB[j + 1], :].rearrange(
                                "u (o s) -> o u s", o=F2, s=SC),
                            ysl[32 * j:32 * j + 32, 0:bw, :])
                    ha = h1p.tile([VA, F2 * SC], BF, tag="ha")
                    hb = h1p.tile([VB, F2 * SC], BF, tag="hb")
                    nc.sync.dma_start(ha[:], h1d[0:VA, :])
                    nc.sync.dma_start(hb[:], h1d[VA:V, :])
                    return ha, hb

                pend = []
                hcur = l2_head(0)
                for c in range(NCH):
                    xsd2 = xdp2.tile([(K - 1) * F2, V * SC], BF, tag="xsd2")
                    conv_spans(c, hcur, F2, xsp2, xsd2, m1ps2, 1)
                    # emit next chunk's head before the lagged projection so
                    # its conv never queues behind the xsT2 read-back
                    if c + 1 < NCH:
                        hcur = l2_head(c + 1)
                    xsT2 = xtp2.tile([(K - 1) * F2, V * SC], BF, tag="xsT2")
                    nc.sync.dma_start(xsT2[:, :], xsd2[:, :])
                    pend.append((c, xsT2))
                    if len(pend) > 2:
                        pc, pxsT = pend.pop(0)
                        proj_evac(pc, stscr2, m2ps2, mm_l2(pxsT))
                for pc, pxsT in pend:
                    proj_evac(pc, stscr2, m2ps2, mm_l2(pxsT))
            bn_finalize(stscr2, gb2t, par2, "2")

            # ---- final normalize + relu + store ----
            with tc.tile_pool(name="stg", bufs=3) as stg:
                for c in range(NCH):
                    sl = c * SC
                    so = stg.tile([128, CW], F32, tag="stg")
                    so3 = so[:, :].rearrange("p (u s) -> p u s", u=42, s=SC)
                    nc.scalar.activation(
                        so[:, :], yslab[:, c * CW:(c + 1) * CW],
                        mybir.ActivationFunctionType.Relu,
                        bias=par2[:, 1:2], scale=par2[:, 0:1])
                    for j in range(4):
                        u0, u1 = UB[j], UB[j + 1]
                        eng = (nc.gpsimd, nc.sync, nc.scalar,
                               nc.gpsimd)[j]
                        eng.dma_start(
                            out[:, u0:u1, sl:sl + SC],
                            so3[32 * j:32 * j + 32, 0:u1 - u0, :])
    nc.compile()
    return nc


def _host_prep(x, lap, w1, w2, g1, be1, g2, be2):
    lap64 = np.asarray(lap).astype(np.float64)
    T = [np.eye(V), lap64]
    for _ in range(2, K):
        T.append(2.0 * lap64 @ T[-1] - T[-2])
    tsk = np.concatenate([T[k].T for k in range(1, K)], axis=1)  # [162, 648]
    w1r = np.asarray(w1).reshape(K * F1, F2)
    w2r = np.asarray(w2).reshape(K * F2, F2)
    w2k14 = w2r[F2:]                       # k=1..4 rows [128, 32]
    w2k0q = np.tile(w2r[0:F2], (4, 1))     # k=0 rows replicated per band
    gb1 = np.stack([np.tile(np.asarray(g1), 4), np.tile(np.asarray(be1), 4)],
                   axis=1)
    gb2 = np.stack([np.tile(np.asarray(g2), 4), np.tile(np.asarray(be2), 4)],
                   axis=1)
    # per-row weight: n_row / total; rows 32j+o weigh band j
    nrow = np.repeat(np.array(BW, np.float64) * S, 32)
    denom = (1.0 if os.environ.get("K_SKIP_CC", "0") == "1" else 8.0) * V * S
    wrow = (nrow / denom).astype(np.float32)[:, None]
    common = {
        "tsk": tsk.astype(BF16),
        "w1r": w1r.astype(BF16),
        "w2k14": w2k14.astype(BF16), "w2k0q": w2k0q.astype(BF16),
        "gb1": gb1.astype(np.float32), "gb2": gb2.astype(np.float32),
        "wrow": wrow,
    }
    in_maps = []
    xf = np.asarray(x).reshape(2, F1, V, 4096)
    for core in range(N_CORES):
        b, q = core // 4, core % 4
        xs = xf[b, :, :, q * S:(q + 1) * S]            # [16, 162, 1024]
        xkv = xs.transpose(1, 0, 2).reshape(V, F1, NCH, SC)
        xkv = xkv.transpose(0, 2, 1, 3)                # [162, 8, 16, 128]
        xktv = xs.reshape(F1, V, NCH, SC).transpose(2, 0, 1, 3)
        m = dict(common)
        m["xk"] = np.ascontiguousarray(xkv).astype(BF16)
        m["xkt"] = np.ascontiguousarray(xktv).reshape(
            NCH, F1, V * SC).astype(BF16)
        in_maps.append(m)
    return in_maps


_CACHE = {}


def _run(in_maps, trace=False):
    if "nc" not in _CACHE:
        _CACHE["nc"] = build_program()
    return run_bass_kernel_spmd(
        _CACHE["nc"], in_maps, core_ids=list(range(N_CORES)), trace=trace)


def kernel(x, lap, w1, b1, g1, be1, w2, b2, g2, be2, _trace=False):
    # conv biases b1/b2 cancel exactly inside BatchNorm -> ignored
    in_maps = _host_prep(x, lap, w1, w2, g1, be1, g2, be2)
    res = _run(in_maps, trace=_trace)
    _CACHE["last_results"] = res
    full = np.empty((2, F2, V, 4096), np.float32)
    for core in range(N_CORES):
        b, q = core // 4, core % 4
        full[b, :, :, q * S:(q + 1) * S] = res.results[core]["out"]
    return full.reshape(2, F2, V, 16, 16, 16)
